# revision 1
# baseline (speedup 1.0000x reference)
"""GCN autoencoder (6x gcn_layer) on 8 TRN2 NeuronCores.

Strategy:
  - Rows of adj_/X sharded across 8 cores; weights replicated.
  - All device tensors bf16 (fp32 PSUM accumulation); host does the free
    sharding / transposes / casts and the final gather+transpose.
  - adj-mm produces zT = (adj_shard @ H)^T so the next layer's XW matmul
    consumes it directly (no transposes anywhere on device).
  - Each layer computes two row-phases (512 local rows each).  After a
    phase: XW(l+1) for those rows -> DRAM bounce -> AllGather -> next
    layer's H chunks; the consumer accumulates its 64 k-chunks in
    arrival-wave order so the second gather's flight hides under the
    first wave's matmuls.
  - Gathered-H / adj-resident / H1 buffers are split per-wave / quartered
    so a reader only depends on the writes that produced its chunk.
  - adj columns 0:512 SBUF-resident (the full bf16 shard does not fit
    beside the H buffers); 512:1024 streamed per layer in k-chunk pairs.
  - Layer 1's H1 = X @ W1 is computed fully on every core from the
    (replicated, free) input X -> no collective before the first adj-mm.
  - Two small warmup AllGathers absorb the collective stream's first-use
    cost while the CC queue is otherwise idle.
  (The ZG z-gather path is disabled: building H locally from a gathered
   z deepened the post-landing critical chain and measured slower.)
"""

import sys

import numpy as np

if "/opt/trn_rl_repo" not in sys.path:
    sys.path.insert(0, "/opt/trn_rl_repo")

import ml_dtypes

import concourse.bacc as bacc
import concourse.tile as tile
from concourse import mybir
from concourse.bass_utils import run_bass_kernel_spmd

N = 8192
D_IN = 512
NCORES = 8
R = N // NCORES  # 1024 rows per core
DIMS = [(512, 256), (256, 256), (256, 128), (128, 256), (256, 256), (256, 512)]

BF16 = mybir.dt.bfloat16
F32 = mybir.dt.float32
NP_BF16 = ml_dtypes.bfloat16
RELU = mybir.ActivationFunctionType.Relu

KO = N // 128  # 64 k-chunks over the gather dim
RT = R // 128  # 8 local row tiles
NPH = 2
PH = R // NPH  # 512 rows per phase
HALF = RT // NPH  # 4 chunks each core contributes per phase
ZG = -1  # disabled: z-gather deepened the post-landing critical chain  # layer index (0-based) whose H is built locally from gathered z

_CACHED = {}


def _build():
    nc = bacc.Bacc(
        "TRN2",
        target_bir_lowering=False,
        debug=False,
        enable_asserts=False,
        num_devices=NCORES,
    )

    adjT = nc.dram_tensor("adjT", [N, R], BF16, kind="ExternalInput")
    xT = nc.dram_tensor("xT", [D_IN, N], BF16, kind="ExternalInput")
    w_dram = [
        nc.dram_tensor(f"W{i + 1}", list(DIMS[i]), BF16, kind="ExternalInput")
        for i in range(6)
    ]
    outT = nc.dram_tensor("outT", [DIMS[-1][1], R], F32, kind="ExternalOutput")

    adjT_r = adjT.ap().rearrange("(ko p) r -> p ko r", p=128)
    xT_r = xT.ap().rearrange("(kx p) c -> p kx c", p=128)

    with tile.TileContext(nc) as tc:
        with (
            tc.tile_pool(name="adjres", bufs=1) as adjres_p,
            tc.tile_pool(name="adjstr", bufs=5) as adjstr_p,
            tc.tile_pool(name="wp", bufs=1) as wp,
            tc.tile_pool(name="xtp", bufs=3) as xtp,
            tc.tile_pool(name="ztgp", bufs=3) as ztgp,
            tc.tile_pool(name="ztp", bufs=8) as ztp,
            tc.tile_pool(name="hp", bufs=6) as hp,
            tc.tile_pool(name="hstage", bufs=4) as hstage,
            tc.tile_pool(name="ostage", bufs=2) as ostage,
            tc.tile_pool(name="psz", bufs=6, space="PSUM") as psz,
            tc.tile_pool(name="psh", bufs=2, space="PSUM") as psh,
            tc.tile_pool(name="dram", bufs=1, space="DRAM") as dram,
        ):
            # ---- resident weights ----
            w_sb = []
            for i, (di, do) in enumerate(DIMS):
                w_t = wp.tile([128, di // 128, do], BF16, name=f"w{i}_sb")
                nc.sync.dma_start(
                    w_t[:], w_dram[i].ap().rearrange("(kx p) n -> p kx n", p=128)
                )
                w_sb.append(w_t)

            # warmup AllGathers: absorb the collective-stream first-use cost
            # while the CC queue is otherwise idle (overlaps XW1 / barrier)
            for wi, wrows in enumerate((16, PH)):
                wu_in = dram.tile([wrows, 256], BF16, tag=f"wu{wi}i",
                                  name=f"wu{wi}i")
                wu_out = dram.tile([NCORES * wrows, 256], BF16,
                                   addr_space="Shared", tag=f"wu{wi}o",
                                   name=f"wu{wi}o")
                nc.gpsimd.collective_compute(
                    "AllGather",
                    mybir.AluOpType.bypass,
                    ins=[wu_in[:].opt()],
                    outs=[wu_out[:].opt()],
                    replica_groups=[list(range(NCORES))],
                )

            # resident adj columns 0:512, quartered so early k-chunk reads
            # only wait on their quarter's DMA; 512:1024 streamed per layer
            adj_res = [
                adjres_p.tile([128, 16, PH], BF16, name=f"adj_res{q}")
                for q in range(4)
            ]
            adj_stream_cache = {}

            def adj_mov(g, n):
                if n == 0:
                    return adj_res[g // 16][:, g % 16, :]
                # pairs: every consumption segment (waves and the
                # half-wave insert) covers complete g//2 pairs, so a pair's
                # pool slot is never revisited after its segment
                grp = g // 2
                t = adj_stream_cache.get(grp)
                if t is None:
                    t = adjstr_p.tile([128, 2, PH], BF16, tag="adjs",
                                      name=f"as{grp}")
                    nc.sync.dma_start(
                        t[:], adjT_r[:, grp * 2 : grp * 2 + 2, PH:R]
                    )
                    adj_stream_cache[grp] = t
                return t[:, g % 2, :]

            # ---- layer 1: H1 = X @ W1 computed fully on every core ----
            # quartered: [128, 16, 256] x4; read of chunk g -> quarter g//16
            h1 = [
                hp.tile([128, 16, DIMS[0][1]], BF16, tag="h", name=f"h1_{q}")
                for q in range(4)
            ]
            for g0 in range(0, KO, 2):
                xt_t = xtp.tile([128, D_IN // 128, 256], BF16, tag="xt")
                nc.sync.dma_start(xt_t[:], xT_r[:, :, g0 * 128 : g0 * 128 + 256])
                for g in (g0, g0 + 1):
                    ps_h = psh.tile([128, DIMS[0][1]], F32, tag="psh")
                    for kx in range(D_IN // 128):
                        c = (g - g0) * 128
                        nc.tensor.matmul(
                            ps_h[:],
                            xt_t[:, kx, c : c + 128],
                            w_sb[0][:, kx, :],
                            start=(kx == 0),
                            stop=(kx == D_IN // 128 - 1),
                        )
                    nc.vector.tensor_copy(h1[g // 16][:, g % 16, :], ps_h[:])

            def h1_read(m, g):
                return h1[g // 16][:, g % 16, m * 128 : (m + 1) * 128]

            h_read = h1_read

            # resident-adj load, emitted after the XW1 stream so the small
            # xT/W DMAs get the queues first; k-ordered to match consumption
            for q in range(4):
                for j in range(0, 16, 4):
                    nc.sync.dma_start(
                        adj_res[q][:, j : j + 4, :],
                        adjT_r[:, q * 16 + j : q * 16 + j + 4, 0:PH],
                    )

            # consumption waves: layer 1 in production order (g ascending);
            # layers >=2 by producer phase ({c*8 + n*4 + j, j<4} per phase n)
            waves_l1 = [list(range(KO // 2)), list(range(KO // 2, KO))]
            waves_g = [
                [c * RT + n * HALF + j
                 for c in range(NCORES) for j in range(HALF)]
                for n in range(NPH)
            ]

            z_gaths = {}  # producer phase n -> gathered zT DRAM buffer

            for li, (di, do) in enumerate(DIMS):
                last = li == len(DIMS) - 1
                mt = do // 128
                kwaves = waves_l1 if li == 0 else waves_g
                adj_stream_cache.clear()
                gather_z = (li + 1 == ZG)  # this layer's output z is gathered

                if not last:
                    di2, do2 = DIMS[li + 1]
                    kxn2 = di2 // 128  # == mt
                    # per-wave (and per-column-half for do2=512) H buffers:
                    # h_next[ci][w] holds chunks {c*8 + w*4 + j} at pos c*4+j
                    ncs = 1 if do2 <= 256 else 2
                    dc2 = do2 if do2 <= 256 else 256
                    h_next = [
                        [hp.tile([128, KO // 2, dc2], BF16, tag="h",
                                 name=f"h{li + 2}_{ci}_{w}")
                         for w in range(NPH)]
                        for ci in range(ncs)
                    ]

                    def make_reader(h_tiles, split):
                        def rd(m, g):
                            ci, mc = (m // 2, m % 2) if split else (0, m)
                            c, r8 = g // 8, g % 8
                            w, j = r8 // 4, r8 % 4
                            return h_tiles[ci][w][:, c * 4 + j,
                                                  mc * 128 : (mc + 1) * 128]
                        return rd

                def build_h_wave(w):
                    # this layer's H chunks for wave w = gathered_z @ W,
                    # computed locally as the wave's gather lands
                    gz_r = z_gaths[w].rearrange(
                        "(c kx p) r -> c p kx r", c=NCORES, p=128
                    )
                    kxn = di // 128
                    for c in range(NCORES):
                        ztg = ztgp.tile([128, kxn, PH], BF16, tag="ztg",
                                        name=f"ztg{w}_{c}")
                        nc.gpsimd.dma_start(ztg[:], gz_r[c])
                        for j in range(HALF):
                            ps_hx = psh.tile([128, do], F32, tag="psh")
                            for kx in range(kxn):
                                nc.tensor.matmul(
                                    ps_hx[:],
                                    ztg[:, kx, j * 128 : (j + 1) * 128],
                                    w_sb[li][:, kx, :],
                                    start=(kx == 0),
                                    stop=(kx == kxn - 1),
                                )
                            for ci in range(len(h_tiles_cur)):
                                c0 = ci * 256
                                dc = min(256, do - c0)
                                nc.vector.tensor_copy(
                                    h_tiles_cur[ci][w][:, c * 4 + j, :],
                                    ps_hx[:, c0 : c0 + dc],
                                )

                ps_zs = [[psz.tile([128, PH], F32, tag="psz", name=f"psz{n}_{m}")
                          for m in range(mt)] for n in range(NPH)]
                mm_cnt = [[0] * mt for _ in range(NPH)]

                def emit_block(wb, n, lo=0, hi=None):
                    for g in kwaves[wb][lo:hi]:
                        mov = adj_mov(g, n)
                        for m in range(mt):
                            nc.tensor.matmul(
                                ps_zs[n][m][:],
                                h_read(m, g),
                                mov,
                                start=(mm_cnt[n][m] == 0),
                                stop=(mm_cnt[n][m] == KO - 1),
                            )
                            mm_cnt[n][m] += 1

                def emit_epilogue(n):
                    zt_p = []
                    for m in range(mt):
                        if last:
                            o_st = ostage.tile([128, PH], F32, tag="ost")
                            nc.scalar.activation(o_st[:], ps_zs[n][m][:], RELU)
                            nc.sync.dma_start(
                                outT[m * 128 : (m + 1) * 128,
                                     n * PH : (n + 1) * PH],
                                o_st[:],
                            )
                        elif gather_z:
                            z_st = hstage.tile([128, PH], BF16, tag="hst")
                            nc.scalar.activation(z_st[:], ps_zs[n][m][:], RELU)
                            nc.scalar.dma_start(
                                zbounce[m * 128 : (m + 1) * 128, :], z_st[:]
                            )
                        else:
                            z_t = ztp.tile([128, PH], BF16, tag="zt",
                                           name=f"z{li + 1}_{m}_{n}")
                            nc.scalar.activation(z_t[:], ps_zs[n][m][:], RELU)
                            zt_p.append(z_t)
                    if last:
                        return
                    if gather_z:
                        # gather zT itself; the consumer builds H locally
                        gz = dram.tile(
                            [NCORES * do, PH], BF16, addr_space="Shared",
                            tag=f"zg{n}", name=f"zg{n}",
                        )
                        nc.gpsimd.collective_compute(
                            "AllGather",
                            mybir.AluOpType.bypass,
                            ins=[zbounce[:].opt()],
                            outs=[gz[:].opt()],
                            replica_groups=[list(range(NCORES))],
                        )
                        z_gaths[n] = gz
                        return
                    # XW(l+1) for this phase's rows -> bounce -> AllGather
                    bounce = dram.tile([PH, do2], BF16, tag=f"hb{li}_{n}",
                                       name=f"hb{li}_{n}")
                    for j in range(HALF):
                        ps_h = psh.tile([128, do2], F32, tag="psh")
                        for kx in range(kxn2):
                            nc.tensor.matmul(
                                ps_h[:],
                                zt_p[kx][:, j * 128 : (j + 1) * 128],
                                w_sb[li + 1][:, kx, :],
                                start=(kx == 0),
                                stop=(kx == kxn2 - 1),
                            )
                        h_st = hstage.tile([128, do2], BF16, tag="hst")
                        nc.vector.tensor_copy(h_st[:], ps_h[:])
                        nc.sync.dma_start(
                            bounce[j * 128 : (j + 1) * 128, :], h_st[:]
                        )
                    gath = dram.tile(
                        [NCORES * PH, do2], BF16, addr_space="Shared",
                        tag=f"hg{li}_{n}", name=f"hg{li}_{n}",
                    )
                    nc.gpsimd.collective_compute(
                        "AllGather",
                        mybir.AluOpType.bypass,
                        ins=[bounce[:].opt()],
                        outs=[gath[:].opt()],
                        replica_groups=[list(range(NCORES))],
                    )
                    g_r = gath.rearrange("(q p) d -> p q d", p=128)
                    for ci in range(len(h_next)):
                        c0 = ci * 256
                        dc = min(256, do2 - c0)
                        for c in range(NCORES):
                            nc.sync.dma_start(
                                h_next[ci][n][:, c * HALF : (c + 1) * HALF, :],
                                g_r[:, c * HALF : (c + 1) * HALF, c0 : c0 + dc],
                            )

                if gather_z:
                    zbounce = dram.tile([do, PH], BF16, tag="zb0",
                                        name=f"zb{li}_0")
                if li == ZG:
                    build_h_wave(0)
                emit_block(0, 0)
                if li == ZG:
                    build_h_wave(1)
                insert = 0 < li < len(DIMS) - 1 and mt <= 2
                if insert:
                    # phase n1's first wave-0 chunks slot in where phase n0
                    # would otherwise stall on the wave-1 gather landing
                    emit_block(0, 1, 0, 16)
                emit_block(1, 0)
                emit_epilogue(0)
                if gather_z:
                    zbounce = dram.tile([do, PH], BF16, tag="zb1",
                                        name=f"zb{li}_1")
                emit_block(0, 1, 16 if insert else 0, None)
                emit_block(1, 1)
                emit_epilogue(1)

                if not last:
                    h_tiles_cur = h_next
                    h_read = make_reader(h_next, len(h_next) > 1)

    nc.compile()
    return nc


def kernel(**inputs):
    X = np.asarray(inputs["X"], dtype=np.float32)
    adj = np.asarray(inputs["adj_"], dtype=np.float32)

    if "nc" not in _CACHED:
        _CACHED["nc"] = _build()
    nc = _CACHED["nc"]

    xT_full = np.ascontiguousarray(X.T).astype(NP_BF16)
    ws = [np.asarray(inputs[f"W{j + 1}"], np.float32).astype(NP_BF16) for j in range(6)]
    in_maps = []
    for i in range(NCORES):
        rows = slice(i * R, (i + 1) * R)
        m = {
            "adjT": np.ascontiguousarray(adj[rows, :].T).astype(NP_BF16),
            "xT": xT_full,
        }
        for j in range(6):
            m[f"W{j + 1}"] = ws[j]
        in_maps.append(m)

    res = run_bass_kernel_spmd(nc, in_maps, core_ids=list(range(NCORES)))
    out = np.concatenate(
        [np.asarray(r["outT"], dtype=np.float32).T for r in res.results], axis=0
    )
    return out



# revision 11
# speedup vs baseline: 1.2917x; 1.2917x over previous
"""GCN autoencoder (6x gcn_layer) on 8 TRN2 NeuronCores — fp8 DoubleRow.

Strategy (v2):
  - Rows of adj_/X sharded across 8 cores; weights replicated.
  - adj stored fp8e4 (host-scaled x8192, values in [0,1)): the FULL 8MB
    shard is SBUF-resident (no per-layer streaming), and the adj-mm runs
    in DoubleRow perf mode (2 fp8 k-chunks per matmul, ~1.4x bf16 rate).
  - H (= z @ W) is cast to fp8e4 per layer with a power-of-2 scale beta_l
    folded into the replicated weights host-side; activations apply
    scale 1/(8192*beta_l) so z is true-scale bf16 every layer.
  - fp8 H rounding leaves a column-sum bias that the adjacency averaging
    cannot suppress; it is removed exactly via dcol = colsum(Hq) -
    colsum(H), computed on device with FD=1 matmuls (ones / negated
    z-colsum moving operands), AllReduce'd, and folded into the existing
    activation's per-partition bias operand: relu(s*psz - 0.5*s*dcol).
  - adj-mm produces zT = (adj_shard @ H)^T so the next layer's XW matmul
    consumes it directly; XW matmuls stay bf16 (z bf16 x W bf16).
  - Two row-phases of 512 rows each per layer; after a phase: XW(l+1) ->
    fp8 -> DRAM bounce -> AllGather (half the bytes of bf16) -> next
    layer's H chunks, consumed in arrival-wave order.
  - Layer 1's H1 = X @ W1 computed fully on every core from the
    replicated input X -> no collective before the first adj-mm.
"""

import sys

import numpy as np

if "/opt/trn_rl_repo" not in sys.path:
    sys.path.insert(0, "/opt/trn_rl_repo")

import ml_dtypes

import concourse.bacc as bacc
import concourse.tile as tile
from concourse import mybir
from concourse.bass_utils import run_bass_kernel_spmd

N = 8192
D_IN = 512
NCORES = 8
R = N // NCORES  # 1024 rows per core
DIMS = [(512, 256), (256, 256), (256, 128), (128, 256), (256, 256), (256, 512)]

ASCALE = 8192.0
BETA = [4.0, 1024.0, 4096.0, 16384.0, 65536.0, 262144.0]
SL = [1.0 / (ASCALE * b) for b in BETA]

BF16 = mybir.dt.bfloat16
F32 = mybir.dt.float32
FP8 = mybir.dt.float8e4
NP_BF16 = ml_dtypes.bfloat16
NP_FP8 = ml_dtypes.float8_e4m3
RELU = mybir.ActivationFunctionType.Relu
DR = mybir.MatmulPerfMode.DoubleRow

KO = N // 128  # 64 k-chunks over the gather dim
KP = KO // 2  # 32 DoubleRow k-pairs
RT = R // 128  # 8 local row tiles
NPH = 2
PH = R // NPH  # 512 rows per phase
HALF = RT // NPH  # 4 chunks each core contributes per phase

_CACHED = {}


def _build():
    nc = bacc.Bacc(
        "TRN2",
        target_bir_lowering=False,
        debug=False,
        enable_asserts=False,
        num_devices=NCORES,
    )

    adjT = nc.dram_tensor("adjT", [N, R], FP8, kind="ExternalInput")
    xT = nc.dram_tensor("xT", [D_IN, N], BF16, kind="ExternalInput")
    w_dram = [
        nc.dram_tensor(f"W{i + 1}", list(DIMS[i]), BF16, kind="ExternalInput")
        for i in range(6)
    ]
    # negated column-sum of X (bf16), for layer 1's dcol correction
    csxn = nc.dram_tensor("csxn", [D_IN, 1], BF16, kind="ExternalInput")
    outT = nc.dram_tensor("outT", [DIMS[-1][1], R], F32, kind="ExternalOutput")

    adjT_r = adjT.ap().rearrange("(ko p) r -> p ko r", p=128)
    xT_r = xT.ap().rearrange("(kx p) c -> p kx c", p=128)

    with tile.TileContext(nc) as tc:
        with (
            tc.tile_pool(name="adjres", bufs=1) as adjres_p,
            tc.tile_pool(name="wp", bufs=1) as wp,
            tc.tile_pool(name="xtp", bufs=3) as xtp,
            tc.tile_pool(name="ztp", bufs=8) as ztp,
            tc.tile_pool(name="hp", bufs=6) as hp,
            tc.tile_pool(name="hstage", bufs=6) as hstage,
            tc.tile_pool(name="ostage", bufs=2) as ostage,
            tc.tile_pool(name="czp", bufs=6) as czp,
            tc.tile_pool(name="dcs", bufs=6) as dcs_p,
            tc.tile_pool(name="biasp", bufs=6) as biasp,
            tc.tile_pool(name="psz", bufs=6, space="PSUM") as psz,
            tc.tile_pool(name="psh", bufs=2, space="PSUM") as psh,
            tc.tile_pool(name="dram", bufs=1, space="DRAM") as dram,
        ):
            # ---- resident weights + small constants ----
            w_sb = []
            for i, (di, do) in enumerate(DIMS):
                w_t = wp.tile([128, di // 128, do], BF16, name=f"w{i}_sb")
                nc.sync.dma_start(
                    w_t[:], w_dram[i].ap().rearrange("(kx p) n -> p kx n", p=128)
                )
                w_sb.append(w_t)
            ones_sb = wp.tile([128, 1], FP8, name="ones_sb")
            nc.vector.memset(ones_sb[:], 1.0)
            csxn_sb = wp.tile([128, D_IN // 128, 1], BF16, name="csxn_sb")
            nc.sync.dma_start(
                csxn_sb[:], csxn.ap().rearrange("(kx p) one -> p kx one", p=128)
            )

            # warmup AllGathers: absorb the collective-stream first-use cost
            # while the CC queue is otherwise idle (overlaps XW1 / barrier)
            for wi, wrows in enumerate((16, PH)):
                wu_in = dram.tile([wrows, 256], BF16, tag=f"wu{wi}i",
                                  name=f"wu{wi}i")
                wu_out = dram.tile([NCORES * wrows, 256], BF16,
                                   addr_space="Shared", tag=f"wu{wi}o",
                                   name=f"wu{wi}o")
                nc.gpsimd.collective_compute(
                    "AllGather",
                    mybir.AluOpType.bypass,
                    ins=[wu_in[:].opt()],
                    outs=[wu_out[:].opt()],
                    replica_groups=[list(range(NCORES))],
                )

            # ---- layer 1: H1 = X @ W1 computed fully on every core ----
            # quartered: [128, 16, 256] x4; read of chunk g -> quarter g//16
            h1 = [
                hp.tile([128, 16, DIMS[0][1]], FP8, tag="h", name=f"h1_{q}")
                for q in range(4)
            ]
            for g0 in range(0, KO, 2):
                xt_t = xtp.tile([128, D_IN // 128, 256], BF16, tag="xt")
                nc.sync.dma_start(xt_t[:], xT_r[:, :, g0 * 128 : g0 * 128 + 256])
                for g in (g0, g0 + 1):
                    ps_h = psh.tile([128, DIMS[0][1]], F32, tag="psh")
                    for kx in range(D_IN // 128):
                        c = (g - g0) * 128
                        nc.tensor.matmul(
                            ps_h[:],
                            xt_t[:, kx, c : c + 128],
                            w_sb[0][:, kx, :],
                            start=(kx == 0),
                            stop=(kx == D_IN // 128 - 1),
                        )
                    nc.vector.tensor_copy(h1[g // 16][:, g % 16, :], ps_h[:])

            def h1_read(m, g0):
                return h1[g0 // 16][:, g0 % 16 : g0 % 16 + 2,
                                    m * 128 : (m + 1) * 128]

            h_read = h1_read

            # full resident adj shard (fp8), k-ordered to match consumption
            adj_res = [
                adjres_p.tile([128, 16, R], FP8, name=f"adj_res{q}")
                for q in range(4)
            ]
            for q in range(4):
                for j in range(0, 16, 4):
                    nc.sync.dma_start(
                        adj_res[q][:, j : j + 4, :],
                        adjT_r[:, q * 16 + j : q * 16 + j + 4, 0:R],
                    )

            def adj_pair(g0, n):
                return adj_res[g0 // 16][:, g0 % 16 : g0 % 16 + 2,
                                         n * PH : (n + 1) * PH]

            # layer 1 dcol: colsum(H1q) - colsum(H1) = colsum(H1q) + csxn@W1
            # (csxn pre-negated on host).  Each dcol column accumulates in
            # its own transient whole-bank psum tile: a matmul with
            # start=True zeroes the entire 2KB PSUM region, so sibling
            # columns cannot share a bank across accumulation groups.
            mt1 = DIMS[0][1] // 128
            dc1 = dcs_p.tile([128, mt1], F32, tag="dcs", name="dc1")
            for mi in range(mt1):
                ps_c_t = psh.tile([128, DIMS[0][1]], F32, tag="psh")
                ps_c = ps_c_t[:, 0:1]
                for q in range(4):
                    for jj in range(16):
                        nc.tensor.matmul(
                            ps_c,
                            h1[q][:, jj, mi * 128 : (mi + 1) * 128],
                            ones_sb[:],
                            start=(q == 0 and jj == 0),
                            stop=False,
                        )
                for kx in range(D_IN // 128):
                    nc.tensor.matmul(
                        ps_c,
                        w_sb[0][:, kx, mi * 128 : (mi + 1) * 128],
                        csxn_sb[:, kx, :],
                        start=False,
                        stop=(kx == D_IN // 128 - 1),
                    )
                nc.vector.tensor_copy(dc1[:, mi : mi + 1], ps_c)
            bias_cur = biasp.tile([128, mt1], F32, tag="bias", name="bias1")
            nc.vector.tensor_scalar_mul(bias_cur[:], dc1[:], -0.5 * SL[0])

            # consumption waves in DoubleRow PAIRS (g0 = even chunk index):
            # layer 1 in production order; layers >=2 by producer phase
            pwaves_l1 = [list(range(0, KO // 2, 2)), list(range(KO // 2, KO, 2))]
            pwaves_g = [
                [c * RT + n * HALF + j0
                 for c in range(NCORES) for j0 in (0, 2)]
                for n in range(NPH)
            ]

            for li, (di, do) in enumerate(DIMS):
                last = li == len(DIMS) - 1
                mt = do // 128
                kwaves = pwaves_l1 if li == 0 else pwaves_g

                if not last:
                    di2, do2 = DIMS[li + 1]
                    kxn2 = di2 // 128  # == mt
                    mt2 = do2 // 128
                    # per-wave (and per-column-half for do2=512) H buffers:
                    # h_next[ci][w] holds chunks {c*8 + w*4 + j} at pos c*4+j
                    ncs = 1 if do2 <= 256 else 2
                    dc2 = do2 if do2 <= 256 else 256
                    h_next = [
                        [hp.tile([128, KO // 2, dc2], FP8, tag="h",
                                 name=f"h{li + 2}_{ci}_{w}")
                         for w in range(NPH)]
                        for ci in range(ncs)
                    ]

                    def make_reader(h_tiles, split):
                        def rd(m, g0):
                            ci, mc = (m // 2, m % 2) if split else (0, m)
                            c, r8 = g0 // 8, g0 % 8
                            w, j0 = r8 // 4, r8 % 4
                            return h_tiles[ci][w][:, c * 4 + j0 : c * 4 + j0 + 2,
                                                  mc * 128 : (mc + 1) * 128]
                        return rd

                    dc_ph = [None, None]  # per-phase dcol partials (SBUF)

                ps_zs = [[psz.tile([128, PH], F32, tag="psz", name=f"psz{n}_{m}")
                          for m in range(mt)] for n in range(NPH)]
                mm_cnt = [[0] * mt for _ in range(NPH)]

                def emit_block(wb, n, lo=0, hi=None):
                    for g0 in kwaves[wb][lo:hi]:
                        mov = adj_pair(g0, n)
                        for m in range(mt):
                            nc.tensor.matmul(
                                ps_zs[n][m][:],
                                h_read(m, g0),
                                mov,
                                start=(mm_cnt[n][m] == 0),
                                stop=(mm_cnt[n][m] == KP - 1),
                                perf_mode=DR,
                            )
                            mm_cnt[n][m] += 1

                def emit_epilogue(n):
                    zt_p = []
                    for m in range(mt):
                        if last:
                            o_st = ostage.tile([128, PH], F32, tag="ost")
                            nc.scalar.activation(
                                o_st[:], ps_zs[n][m][:], RELU,
                                bias=bias_cur[:, m : m + 1], scale=SL[li],
                            )
                            nc.sync.dma_start(
                                outT[m * 128 : (m + 1) * 128,
                                     n * PH : (n + 1) * PH],
                                o_st[:],
                            )
                        else:
                            z_t = ztp.tile([128, PH], BF16, tag="zt",
                                           name=f"z{li + 1}_{m}_{n}")
                            nc.scalar.activation(
                                z_t[:], ps_zs[n][m][:], RELU,
                                bias=bias_cur[:, m : m + 1], scale=SL[li],
                            )
                            zt_p.append(z_t)
                    if last:
                        return
                    # negated z column-sums (moving operands of dcol's
                    # colsum(H) part): ncz[kx] = -sum_rows z[:, kx]
                    ncz = []
                    for kx in range(kxn2):
                        czf = czp.tile([128, 1], F32, tag="czf")
                        nc.vector.tensor_reduce(
                            czf[:], zt_p[kx][:], mybir.AxisListType.X,
                            mybir.AluOpType.add, negate=True,
                        )
                        czt = czp.tile([128, 1], BF16, tag="cz")
                        nc.vector.tensor_copy(czt[:], czf[:])
                        ncz.append(czt)
                    # XW(l+1) for this phase's rows -> bounce -> AllGather
                    bounce = dram.tile([PH, do2], FP8, tag=f"hb{li}_{n}",
                                       name=f"hb{li}_{n}")
                    h_sts = []
                    for j in range(HALF):
                        ps_h = psh.tile([128, do2], F32, tag="psh")
                        for kx in range(kxn2):
                            nc.tensor.matmul(
                                ps_h[:],
                                zt_p[kx][:, j * 128 : (j + 1) * 128],
                                w_sb[li + 1][:, kx, :],
                                start=(kx == 0),
                                stop=(kx == kxn2 - 1),
                            )
                        h_st = hstage.tile([128, do2], FP8, tag="hst")
                        nc.vector.tensor_copy(h_st[:], ps_h[:])
                        nc.sync.dma_start(
                            bounce[j * 128 : (j + 1) * 128, :], h_st[:]
                        )
                        h_sts.append(h_st)
                    # this phase's dcol partial: colsum(Hq) - colsum_z @ W;
                    # one whole-bank psum tile per column (see dc1 comment)
                    dcp = dcs_p.tile([128, mt2], F32, tag="dcs",
                                     name=f"dcp{li}_{n}")
                    dc_ph[n] = dcp
                    for mi in range(mt2):
                        ps_c_t = psh.tile([128, do2], F32, tag="psh")
                        ps_c = ps_c_t[:, 0:1]
                        for j in range(HALF):
                            nc.tensor.matmul(
                                ps_c,
                                h_sts[j][:, mi * 128 : (mi + 1) * 128],
                                ones_sb[:],
                                start=(j == 0),
                                stop=False,
                            )
                        for kx in range(kxn2):
                            nc.tensor.matmul(
                                ps_c,
                                w_sb[li + 1][:, kx, mi * 128 : (mi + 1) * 128],
                                ncz[kx][:],
                                start=False,
                                stop=(kx == kxn2 - 1),
                            )
                        nc.vector.tensor_copy(dcp[:, mi : mi + 1], ps_c)
                    gath = dram.tile(
                        [NCORES * PH, do2], FP8, addr_space="Shared",
                        tag=f"hg{li}_{n}", name=f"hg{li}_{n}",
                    )
                    nc.gpsimd.collective_compute(
                        "AllGather",
                        mybir.AluOpType.bypass,
                        ins=[bounce[:].opt()],
                        outs=[gath[:].opt()],
                        replica_groups=[list(range(NCORES))],
                    )
                    g_r = gath.rearrange("(q p) d -> p q d", p=128)
                    for ci in range(len(h_next)):
                        c0 = ci * 256
                        dc = min(256, do2 - c0)
                        for c in range(NCORES):
                            nc.sync.dma_start(
                                h_next[ci][n][:, c * HALF : (c + 1) * HALF, :],
                                g_r[:, c * HALF : (c + 1) * HALF, c0 : c0 + dc],
                            )
                    if n == 1:
                        # both phases accumulated: AllReduce dcol, build the
                        # next layer's activation bias = -0.5*s*dcol
                        dc_sb = dcs_p.tile([128, mt2], F32, tag="dcs")
                        nc.vector.tensor_tensor(
                            dc_sb[:], dc_ph[0][:], dc_ph[1][:],
                            mybir.AluOpType.add,
                        )
                        dcb = dram.tile([do2, 1], F32, tag=f"dcb{li}",
                                        name=f"dcb{li}")
                        nc.sync.dma_start(
                            dcb.rearrange("(m p) one -> p (m one)", p=128),
                            dc_sb[:],
                        )
                        gdc = dram.tile([do2, 1], F32, addr_space="Shared",
                                        tag=f"gdc{li}", name=f"gdc{li}")
                        nc.gpsimd.collective_compute(
                            "AllReduce",
                            mybir.AluOpType.add,
                            ins=[dcb[:].opt()],
                            outs=[gdc[:].opt()],
                            replica_groups=[list(range(NCORES))],
                        )
                        braw = biasp.tile([128, mt2], F32, tag="bias",
                                          name=f"braw{li}")
                        nc.sync.dma_start(
                            braw[:],
                            gdc.rearrange("(m p) one -> p (m one)", p=128),
                        )
                        bnext = biasp.tile([128, mt2], F32, tag="bias",
                                           name=f"bias{li + 2}")
                        nc.vector.tensor_scalar_mul(
                            bnext[:], braw[:], -0.5 * SL[li + 1]
                        )
                        emit_epilogue.bias_next = bnext

                emit_block(0, 0)
                insert = 0 < li < len(DIMS) - 1 and mt <= 2
                if insert:
                    # phase n1's first wave-0 pairs slot in where phase n0
                    # would otherwise stall on the wave-1 gather landing
                    emit_block(0, 1, 0, 8)
                emit_block(1, 0)
                emit_epilogue(0)
                emit_block(0, 1, 8 if insert else 0, None)
                emit_block(1, 1)
                emit_epilogue(1)

                if not last:
                    h_read = make_reader(h_next, len(h_next) > 1)
                    bias_cur = emit_epilogue.bias_next

    nc.compile()
    return nc


def prepare_in_maps(inputs):
    X = np.asarray(inputs["X"], dtype=np.float32)
    adj = np.asarray(inputs["adj_"], dtype=np.float32)

    xT_full = np.ascontiguousarray(X.T).astype(NP_BF16)
    ws = [
        (np.asarray(inputs[f"W{j + 1}"], np.float32) * BETA[j]).astype(NP_BF16)
        for j in range(6)
    ]
    csxn_full = (
        -X.astype(NP_BF16).astype(np.float32).sum(axis=0, keepdims=True)
    ).astype(NP_BF16).reshape(D_IN, 1)
    adj_s = adj * ASCALE
    in_maps = []
    for i in range(NCORES):
        rows = slice(i * R, (i + 1) * R)
        m = {
            "adjT": np.ascontiguousarray(adj_s[rows, :].T).astype(NP_FP8),
            "xT": xT_full,
            "csxn": csxn_full,
        }
        for j in range(6):
            m[f"W{j + 1}"] = ws[j]
        in_maps.append(m)
    return in_maps


def kernel(**inputs):
    if "nc" not in _CACHED:
        _CACHED["nc"] = _build()
    nc = _CACHED["nc"]

    in_maps = prepare_in_maps(inputs)
    res = run_bass_kernel_spmd(nc, in_maps, core_ids=list(range(NCORES)))
    out = np.concatenate(
        [np.asarray(r["outT"], dtype=np.float32).T for r in res.results], axis=0
    )
    return out


# revision 16
# speedup vs baseline: 1.3740x; 1.0638x over previous
"""GCN autoencoder (6x gcn_layer) on 8 TRN2 NeuronCores — fp8 DoubleRow.

Strategy (v2):
  - Rows of adj_/X sharded across 8 cores; weights replicated.
  - adj stored fp8e4 (host-scaled x8192, values in [0,1)): the FULL 8MB
    shard is SBUF-resident (no per-layer streaming), and the adj-mm runs
    in DoubleRow perf mode (2 fp8 k-chunks per matmul, ~1.4x bf16 rate).
  - H (= z @ W) is cast to fp8e4 per layer with a power-of-2 scale beta_l
    folded into the replicated weights host-side; activations apply
    scale 1/(8192*beta_l) so z is true-scale bf16 every layer.
  - fp8 H rounding leaves a column-sum bias that the adjacency averaging
    cannot suppress; it is removed exactly via dcol = colsum(Hq) -
    colsum(H), computed on device with FD=1 matmuls (ones / negated
    z-colsum moving operands), AllReduce'd, and folded into the existing
    activation's per-partition bias operand: relu(s*psz - 0.5*s*dcol).
  - adj-mm produces zT = (adj_shard @ H)^T so the next layer's XW matmul
    consumes it directly; XW matmuls stay bf16 (z bf16 x W bf16).
  - Two row-phases of 512 rows each per layer; after a phase: XW(l+1) ->
    fp8 -> DRAM bounce -> AllGather (half the bytes of bf16) -> next
    layer's H chunks, consumed in arrival-wave order.
  - Layer 1's H1 = X @ W1 computed fully on every core from the
    replicated input X -> no collective before the first adj-mm.
"""

import sys

import numpy as np

if "/opt/trn_rl_repo" not in sys.path:
    sys.path.insert(0, "/opt/trn_rl_repo")

import ml_dtypes

import concourse.bacc as bacc
import concourse.tile as tile
from concourse import mybir
from concourse.bass_utils import run_bass_kernel_spmd

N = 8192
D_IN = 512
NCORES = 8
R = N // NCORES  # 1024 rows per core
DIMS = [(512, 256), (256, 256), (256, 128), (128, 256), (256, 256), (256, 512)]

ASCALE = 8192.0
BETA = [4.0, 1024.0, 4096.0, 16384.0, 65536.0, 262144.0]
SL = [1.0 / (ASCALE * b) for b in BETA]

BF16 = mybir.dt.bfloat16
F32 = mybir.dt.float32
FP8 = mybir.dt.float8e4
NP_BF16 = ml_dtypes.bfloat16
NP_FP8 = ml_dtypes.float8_e4m3
RELU = mybir.ActivationFunctionType.Relu
DR = mybir.MatmulPerfMode.DoubleRow

KO = N // 128  # 64 k-chunks over the gather dim
KP = KO // 2  # 32 DoubleRow k-pairs
RT = R // 128  # 8 local row tiles
NPH = 2
PH = R // NPH  # 512 rows per phase
HALF = RT // NPH  # 4 chunks each core contributes per phase

_CACHED = {}


def _build():
    nc = bacc.Bacc(
        "TRN2",
        target_bir_lowering=False,
        debug=False,
        enable_asserts=False,
        num_devices=NCORES,
    )

    adjT = nc.dram_tensor("adjT", [N, R], FP8, kind="ExternalInput")
    xT = nc.dram_tensor("xT", [D_IN, N], BF16, kind="ExternalInput")
    w_dram = [
        nc.dram_tensor(f"W{i + 1}", list(DIMS[i]), BF16, kind="ExternalInput")
        for i in range(6)
    ]
    # negated column-sum of X (bf16), for layer 1's dcol correction
    csxn = nc.dram_tensor("csxn", [D_IN, 1], BF16, kind="ExternalInput")
    outT = nc.dram_tensor("outT", [DIMS[-1][1], R], F32, kind="ExternalOutput")

    adjT_r = adjT.ap().rearrange("(ko p) r -> p ko r", p=128)
    xT_r = xT.ap().rearrange("(kx p) c -> p kx c", p=128)

    with tile.TileContext(nc) as tc:
        with (
            tc.tile_pool(name="adjres", bufs=1) as adjres_p,
            tc.tile_pool(name="wp", bufs=1) as wp,
            tc.tile_pool(name="xtp", bufs=3) as xtp,
            tc.tile_pool(name="ztp", bufs=8) as ztp,
            tc.tile_pool(name="hp", bufs=6) as hp,
            tc.tile_pool(name="hstage", bufs=6) as hstage,
            tc.tile_pool(name="ostage", bufs=2) as ostage,
            tc.tile_pool(name="czp", bufs=6) as czp,
            tc.tile_pool(name="dcs", bufs=6) as dcs_p,
            tc.tile_pool(name="biasp", bufs=6) as biasp,
            tc.tile_pool(name="psz", bufs=6, space="PSUM") as psz,
            tc.tile_pool(name="psh", bufs=2, space="PSUM") as psh,
            tc.tile_pool(name="dram", bufs=1, space="DRAM") as dram,
        ):
            # ---- resident weights + small constants ----
            w_sb = []
            for i, (di, do) in enumerate(DIMS):
                w_t = wp.tile([128, di // 128, do], BF16, name=f"w{i}_sb")
                nc.sync.dma_start(
                    w_t[:], w_dram[i].ap().rearrange("(kx p) n -> p kx n", p=128)
                )
                w_sb.append(w_t)
            ones_sb = wp.tile([128, 1], FP8, name="ones_sb")
            nc.vector.memset(ones_sb[:], 1.0)
            csxn_sb = wp.tile([128, D_IN // 128, 1], BF16, name="csxn_sb")
            nc.sync.dma_start(
                csxn_sb[:], csxn.ap().rearrange("(kx p) one -> p kx one", p=128)
            )

            # warmup AllGathers: absorb the collective-stream first-use cost
            # while the CC queue is otherwise idle (overlaps XW1 / barrier)
            for wi, wrows in enumerate((16, PH)):
                wu_in = dram.tile([wrows, 256], BF16, tag=f"wu{wi}i",
                                  name=f"wu{wi}i")
                wu_out = dram.tile([NCORES * wrows, 256], BF16,
                                   addr_space="Shared", tag=f"wu{wi}o",
                                   name=f"wu{wi}o")
                nc.gpsimd.collective_compute(
                    "AllGather",
                    mybir.AluOpType.bypass,
                    ins=[wu_in[:].opt()],
                    outs=[wu_out[:].opt()],
                    replica_groups=[list(range(NCORES))],
                )

            # ---- layer 1: H1 = X @ W1 computed fully on every core ----
            # quartered: [128, 16, 256] x4; read of chunk g -> quarter g//16
            h1 = [
                hp.tile([128, 16, DIMS[0][1]], FP8, tag="h", name=f"h1_{q}")
                for q in range(4)
            ]
            for g0 in range(0, KO, 2):
                xt_t = xtp.tile([128, D_IN // 128, 256], BF16, tag="xt")
                nc.sync.dma_start(xt_t[:], xT_r[:, :, g0 * 128 : g0 * 128 + 256])
                for g in (g0, g0 + 1):
                    ps_h = psh.tile([128, DIMS[0][1]], F32, tag="psh")
                    for kx in range(D_IN // 128):
                        c = (g - g0) * 128
                        nc.tensor.matmul(
                            ps_h[:],
                            xt_t[:, kx, c : c + 128],
                            w_sb[0][:, kx, :],
                            start=(kx == 0),
                            stop=(kx == D_IN // 128 - 1),
                        )
                    nc.vector.tensor_copy(h1[g // 16][:, g % 16, :], ps_h[:])

            def h1_read(m, g0):
                return h1[g0 // 16][:, g0 % 16 : g0 % 16 + 2,
                                    m * 128 : (m + 1) * 128]

            h_read = h1_read

            # full resident adj shard (fp8), k-ordered to match consumption
            adj_res = [
                adjres_p.tile([128, 16, R], FP8, name=f"adj_res{q}")
                for q in range(4)
            ]
            for q in range(4):
                for j in range(0, 16, 4):
                    nc.sync.dma_start(
                        adj_res[q][:, j : j + 4, :],
                        adjT_r[:, q * 16 + j : q * 16 + j + 4, 0:R],
                    )

            def adj_pair(g0, n):
                return adj_res[g0 // 16][:, g0 % 16 : g0 % 16 + 2,
                                         n * PH : (n + 1) * PH]

            # layer 1 dcol: colsum(H1q) - colsum(H1) = colsum(H1q) + csxn@W1
            # (csxn pre-negated on host).  Each dcol column accumulates in
            # its own transient whole-bank psum tile: a matmul with
            # start=True zeroes the entire 2KB PSUM region, so sibling
            # columns cannot share a bank across accumulation groups.
            mt1 = DIMS[0][1] // 128
            dc1 = dcs_p.tile([128, mt1], F32, tag="dcs", name="dc1")
            for mi in range(mt1):
                ps_c_t = psh.tile([128, DIMS[0][1]], F32, tag="psh")
                ps_c = ps_c_t[:, 0:1]
                for q in range(4):
                    for jj in range(16):
                        nc.tensor.matmul(
                            ps_c,
                            h1[q][:, jj, mi * 128 : (mi + 1) * 128],
                            ones_sb[:],
                            start=(q == 0 and jj == 0),
                            stop=False,
                        )
                for kx in range(D_IN // 128):
                    nc.tensor.matmul(
                        ps_c,
                        w_sb[0][:, kx, mi * 128 : (mi + 1) * 128],
                        csxn_sb[:, kx, :],
                        start=False,
                        stop=(kx == D_IN // 128 - 1),
                    )
                nc.vector.tensor_copy(dc1[:, mi : mi + 1], ps_c)
            bias_cur = biasp.tile([128, mt1], F32, tag="bias", name="bias1")
            nc.vector.tensor_scalar_mul(bias_cur[:], dc1[:], -0.5 * SL[0])

            # consumption waves in DoubleRow PAIRS (g0 = even chunk index):
            # layer 1 in production order; layers >=2 by producer phase
            pwaves_l1 = [list(range(0, KO // 2, 2)), list(range(KO // 2, KO, 2))]
            pwaves_g = [
                [c * RT + n * HALF + j0
                 for c in range(NCORES) for j0 in (0, 2)]
                for n in range(NPH)
            ]

            for li, (di, do) in enumerate(DIMS):
                last = li == len(DIMS) - 1
                mt = do // 128
                kwaves = pwaves_l1 if li == 0 else pwaves_g

                if not last:
                    di2, do2 = DIMS[li + 1]
                    kxn2 = di2 // 128  # == mt
                    mt2 = do2 // 128
                    # per-wave (and per-column-half for do2=512) H buffers:
                    # h_next[ci][w] holds chunks {c*8 + w*4 + j} at pos c*4+j
                    ncs = 1 if do2 <= 256 else 2
                    dc2 = do2 if do2 <= 256 else 256
                    h_next = [
                        [hp.tile([128, KO // 2, dc2], FP8, tag="h",
                                 name=f"h{li + 2}_{ci}_{w}")
                         for w in range(NPH)]
                        for ci in range(ncs)
                    ]

                    def make_reader(h_tiles, split):
                        def rd(m, g0):
                            ci, mc = (m // 2, m % 2) if split else (0, m)
                            c, r8 = g0 // 8, g0 % 8
                            w, j0 = r8 // 4, r8 % 4
                            return h_tiles[ci][w][:, c * 4 + j0 : c * 4 + j0 + 2,
                                                  mc * 128 : (mc + 1) * 128]
                        return rd

                    tl_t = dcs_p.tile([128, mt2, 2 * NCORES], F32, tag="tl",
                                      name=f"tl{li}")

                ps_zs = [[psz.tile([128, PH], F32, tag="psz", name=f"psz{n}_{m}")
                          for m in range(mt)] for n in range(NPH)]
                mm_cnt = [[0] * mt for _ in range(NPH)]

                def emit_block(wb, n, lo=0, hi=None):
                    for g0 in kwaves[wb][lo:hi]:
                        mov = adj_pair(g0, n)
                        for m in range(mt):
                            nc.tensor.matmul(
                                ps_zs[n][m][:],
                                h_read(m, g0),
                                mov,
                                start=(mm_cnt[n][m] == 0),
                                stop=(mm_cnt[n][m] == KP - 1),
                                perf_mode=DR,
                            )
                            mm_cnt[n][m] += 1

                def emit_epilogue(n):
                    zt_p = []
                    for m in range(mt):
                        if last:
                            o_st = ostage.tile([128, PH], F32, tag="ost")
                            nc.scalar.activation(
                                o_st[:], ps_zs[n][m][:], RELU,
                                bias=bias_cur[:, m : m + 1], scale=SL[li],
                            )
                            nc.sync.dma_start(
                                outT[m * 128 : (m + 1) * 128,
                                     n * PH : (n + 1) * PH],
                                o_st[:],
                            )
                        else:
                            z_t = ztp.tile([128, PH], BF16, tag="zt",
                                           name=f"z{li + 1}_{m}_{n}")
                            nc.scalar.activation(
                                z_t[:], ps_zs[n][m][:], RELU,
                                bias=bias_cur[:, m : m + 1], scale=SL[li],
                            )
                            zt_p.append(z_t)
                    if last:
                        return
                    # negated z column-sums (moving operands of dcol's
                    # colsum(H) part): ncz[kx] = -sum_rows z[:, kx]
                    ncz = []
                    for kx in range(kxn2):
                        czf = czp.tile([128, 1], F32, tag="czf")
                        nc.vector.tensor_reduce(
                            czf[:], zt_p[kx][:], mybir.AxisListType.X,
                            mybir.AluOpType.add, negate=True,
                        )
                        czt = czp.tile([128, 1], BF16, tag="cz")
                        nc.vector.tensor_copy(czt[:], czf[:])
                        ncz.append(czt)
                    # XW(l+1) for this phase's rows -> bounce -> AllGather.
                    # The bounce carries a 4-row f32 tail: this phase's dcol
                    # partial rides the H gather, so no separate AllReduce
                    # sits on the CC queue's critical chain.
                    bounce = dram.tile([PH + 4, do2], FP8, tag=f"hb{li}_{n}",
                                       name=f"hb{li}_{n}")
                    h_sts = []
                    for j in range(HALF):
                        ps_h = psh.tile([128, do2], F32, tag="psh")
                        for kx in range(kxn2):
                            nc.tensor.matmul(
                                ps_h[:],
                                zt_p[kx][:, j * 128 : (j + 1) * 128],
                                w_sb[li + 1][:, kx, :],
                                start=(kx == 0),
                                stop=(kx == kxn2 - 1),
                            )
                        h_st = hstage.tile([128, do2], FP8, tag="hst")
                        nc.vector.tensor_copy(h_st[:], ps_h[:])
                        nc.sync.dma_start(
                            bounce[j * 128 : (j + 1) * 128, :], h_st[:]
                        )
                        h_sts.append(h_st)
                    # this phase's dcol partial: colsum(Hq) - colsum_z @ W;
                    # one whole-bank psum tile per column (see dc1 comment)
                    dcp = dcs_p.tile([128, mt2], F32, tag="dcs",
                                     name=f"dcp{li}_{n}")
                    for mi in range(mt2):
                        ps_c_t = psh.tile([128, do2], F32, tag="psh")
                        ps_c = ps_c_t[:, 0:1]
                        for j in range(HALF):
                            nc.tensor.matmul(
                                ps_c,
                                h_sts[j][:, mi * 128 : (mi + 1) * 128],
                                ones_sb[:],
                                start=(j == 0),
                                stop=False,
                            )
                        for kx in range(kxn2):
                            nc.tensor.matmul(
                                ps_c,
                                w_sb[li + 1][:, kx, mi * 128 : (mi + 1) * 128],
                                ncz[kx][:],
                                start=False,
                                stop=(kx == kxn2 - 1),
                            )
                        nc.vector.tensor_copy(dcp[:, mi : mi + 1], ps_c)
                    nc.sync.dma_start(
                        bounce[PH : PH + 4, :].bitcast(F32), dcp[:]
                    )
                    gath = dram.tile(
                        [NCORES * (PH + 4), do2], FP8, addr_space="Shared",
                        tag=f"hg{li}_{n}", name=f"hg{li}_{n}",
                    )
                    nc.gpsimd.collective_compute(
                        "AllGather",
                        mybir.AluOpType.bypass,
                        ins=[bounce[:].opt()],
                        outs=[gath[:].opt()],
                        replica_groups=[list(range(NCORES))],
                    )
                    for ci in range(len(h_next)):
                        c0 = ci * 256
                        dc = min(256, do2 - c0)
                        for c in range(NCORES):
                            r0 = c * (PH + 4)
                            nc.sync.dma_start(
                                h_next[ci][n][:, c * HALF : (c + 1) * HALF, :],
                                gath[r0 : r0 + PH, c0 : c0 + dc].rearrange(
                                    "(q p) d -> p q d", p=128
                                ),
                            )
                    for c in range(NCORES):
                        r0 = c * (PH + 4) + PH
                        nc.sync.dma_start(
                            tl_t[:, :, n * NCORES + c],
                            gath[r0 : r0 + 4, :].bitcast(F32),
                        )
                    if n == 1:
                        # all 16 dcol partials landed: reduce, then build the
                        # next layer's activation bias = -0.5*s*dcol
                        dcg = dcs_p.tile([128, mt2], F32, tag="dcs",
                                         name=f"dcg{li}")
                        nc.vector.tensor_reduce(
                            dcg[:], tl_t[:], mybir.AxisListType.X,
                            mybir.AluOpType.add,
                        )
                        bnext = biasp.tile([128, mt2], F32, tag="bias",
                                           name=f"bias{li + 2}")
                        nc.vector.tensor_scalar_mul(
                            bnext[:], dcg[:], -0.5 * SL[li + 1]
                        )
                        emit_epilogue.bias_next = bnext

                if mt <= 2:
                    # all wave-0 work (both phases) before any wave-1 work:
                    # the in-order PE queue then has a full phase of
                    # executable matmuls while the wave-1 gather lands
                    emit_block(0, 0)
                    emit_block(0, 1)
                    emit_block(1, 0)
                    emit_epilogue(0)
                    emit_block(1, 1)
                    emit_epilogue(1)
                else:
                    # mt=4: both phases' groups exceed the psz pool; keep
                    # the phase-serial order
                    emit_block(0, 0)
                    emit_block(1, 0)
                    emit_epilogue(0)
                    emit_block(0, 1)
                    emit_block(1, 1)
                    emit_epilogue(1)

                if not last:
                    h_read = make_reader(h_next, len(h_next) > 1)
                    bias_cur = emit_epilogue.bias_next

    nc.compile()
    return nc


def prepare_in_maps(inputs):
    X = np.asarray(inputs["X"], dtype=np.float32)
    adj = np.asarray(inputs["adj_"], dtype=np.float32)

    xT_full = np.ascontiguousarray(X.T).astype(NP_BF16)
    ws = [
        (np.asarray(inputs[f"W{j + 1}"], np.float32) * BETA[j]).astype(NP_BF16)
        for j in range(6)
    ]
    csxn_full = (
        -X.astype(NP_BF16).astype(np.float32).sum(axis=0, keepdims=True)
    ).astype(NP_BF16).reshape(D_IN, 1)
    adj_s = adj * ASCALE
    in_maps = []
    for i in range(NCORES):
        rows = slice(i * R, (i + 1) * R)
        m = {
            "adjT": np.ascontiguousarray(adj_s[rows, :].T).astype(NP_FP8),
            "xT": xT_full,
            "csxn": csxn_full,
        }
        for j in range(6):
            m[f"W{j + 1}"] = ws[j]
        in_maps.append(m)
    return in_maps


def kernel(**inputs):
    if "nc" not in _CACHED:
        _CACHED["nc"] = _build()
    nc = _CACHED["nc"]

    in_maps = prepare_in_maps(inputs)
    res = run_bass_kernel_spmd(nc, in_maps, core_ids=list(range(NCORES)))
    out = np.concatenate(
        [np.asarray(r["outT"], dtype=np.float32).T for r in res.results], axis=0
    )
    return out
